# revision 31
# baseline (speedup 1.0000x reference)
"""Trainium2 Bass kernel for the CSMHP (clustered self-exciting Hawkes process)
negative log-likelihood, distributed over 8 NeuronCores.

Math
----
The reference builds the full (C, N, N) pairwise decay tensor and row-reduces
it with logsumexp.  The excitation

    E[c, i] = sum_{j<i} exp(-beta_c * (t_i - t_j))

obeys the first-order recurrence  E_i = d_i * (E_{i-1} + 1)  with
d_i = exp(-beta_c * (t_i - t_{i-1})), which maps exactly onto the DVE
`tensor_tensor_scan` instruction: state = (d *mult* state) *add* d.
That turns the O(N^2 C) pairwise tensor into O(N C) work.

Sharding
--------
Events are split into 8 contiguous blocks of 512 (the N axis of the pairwise
tensor, as the hint suggests).  Each core:
  * computes its scan-initial state A'[c] = E[c, first_own_event - 1] directly
    from the (padded, uniform-shape) list of prior events — a dense
    exp+reduce over at most 3584 values, so no cross-core recurrence and no
    collectives are needed;
  * runs the scan over its 512 events for all 8 clusters at once;
  * reduces its partial log-likelihood sum, its partial probability column
    sum, and (core 7) the excitation at the very last event, which is exactly
    the data the analytic integral term needs.
The host sums the 8 partial scalars (the "all-reduce" of the hint).
"""

import numpy as np

import concourse.bass as bass
import concourse.tile as tile
from concourse import mybir
from concourse.bass_utils import run_bass_kernel_spmd
from concourse.vector_clock import ScopedClock

F32 = mybir.dt.float32
ALU = mybir.AluOpType
ACT = mybir.ActivationFunctionType

N = 4096
C = 8
NCORES = 8
CHUNK = N // NCORES          # 512 events per core
PRIOR_PAD = 3584             # max prior events (core 7: 512*7-1=3583), padded
PCOLS = PRIOR_PAD // 128     # 28
T_WINDOW = 100.0
BIG = 1.0e9                  # pad offset: exp(-beta*BIG) == 0 in fp32

_NC_CACHE = None


class SplitDrainTileContext(tile.TileContext):
    """TileContext whose kernel-tail drain carries one sem wait per Drain.

    The TPB instruction encoding in this toolchain holds a single embedded
    semaphore wait; the stock TileContext attaches every outstanding proc
    semaphore to one Drain, which walrus rejects with "Too many sync wait
    commands".  Emitting a chain of Drains (one wait each) on the same SP
    queue is sequentially equivalent.
    """

    def _drain_and_barrier(self, tick_clock, wait_clock):
        drain_inst = self.nc.sync.drain()
        wait_clock.add_sem_waits(
            drain_inst.ins, ScopedClock({None: tick_clock.global_clock})
        )
        si = drain_inst.ins.sync_info
        if si is not None and si.on_wait and len(si.on_wait) > 1:
            waits = list(si.on_wait)
            drain_inst.ins.sync_info = mybir.SyncInfo(
                on_wait=[waits[0]], on_update=list(si.on_update or [])
            )
            for w in waits[1:]:
                extra = self.nc.sync.drain()
                extra.ins.sync_info = mybir.SyncInfo(on_wait=[w], on_update=[])

        self.nc.all_engine_barrier()
        assert self.sems is not None
        popped = self.nc._tile_sem_poison_stack.pop()
        assert popped is self._sem_poison
        self.nc.clear_and_free_semaphores(list(self.sems.allocated().values()))
        self.nc.all_engine_barrier()


def _build_nc():
    """Build the (SPMD-uniform) Bass program run on every core.

    Sync-wait budget: the CoreV3 DVE instruction encoding holds a single
    embedded semaphore wait, so every DVE op may depend on at most ONE
    foreign processor.  All 8-partition inputs travel in one DMA (inA), all
    128-partition inputs in another (inB), and the PSUM matmul result is
    bounced through an ACT copy so the scan's producers are both ACT.
    """
    nc = bass.Bass("TRN2", target_bir_lowering=False, debug=False)

    # inA columns: [0:512] t_own | [512:1024] t_prev | [1024:1536] pT
    #              [1536:1540] scal (beta, alpha, mu, gamma)
    ina_d = nc.dram_tensor("inA", [C, 3 * CHUNK + 4], F32, kind="ExternalInput")
    # inB columns: [0:224] prior_rep (prior tiled x8) | [224:225] tref
    #              [225:449] b128_rep (beta_c per 28-col group) | [449:457] ones
    CP = C * PCOLS  # 224
    inb_d = nc.dram_tensor("inB", [128, 2 * CP + 1 + C], F32, kind="ExternalInput")

    # out columns: 0 = per-core probability column sums, 1 = last-event
    # excitation, 2 = ll partial in row 0
    out_d = nc.dram_tensor("out", [C, 3], F32, kind="ExternalOutput")

    with SplitDrainTileContext(nc) as tc:
        with (
            tc.tile_pool(name="sb", bufs=1) as sb,
            tc.tile_pool(name="ps", bufs=1, space="PSUM") as ps,
        ):
            ina = sb.tile([C, 3 * CHUNK + 4], F32)
            nc.gpsimd.dma_start(out=ina, in_=ina_d.ap())
            inb = sb.tile([128, 2 * CP + 1 + C], F32)
            nc.gpsimd.dma_start(out=inb, in_=inb_d.ap())

            t_own = ina[:, 0:CHUNK]
            t_prev = ina[:, CHUNK : 2 * CHUNK]
            pt = ina[:, 2 * CHUNK : 3 * CHUNK]
            scal = ina[:, 3 * CHUNK : 3 * CHUNK + 4]
            prior_rep = inb[:, 0:CP]
            tref = inb[:, CP : CP + 1]
            b128_rep = inb[:, CP + 1 : 2 * CP + 1]
            ones_in = inb[:, 2 * CP + 1 : 2 * CP + 1 + C]

            beta_col = scal[:, 0:1]
            alpha_col = scal[:, 1:2]
            mu_col = scal[:, 2:3]
            gamma_col = scal[:, 3:4]

            out_stage = sb.tile([C, 3], F32)

            # ---- prolog: A'[c] = sum_j exp(beta_c * (prior_j - t_ref)) ----
            # wbig[p, (c,j)] = (prior[p, j] - tref[p]) * beta_c in one DVE op
            # (prior/beta repeats are host-materialized: contiguous APs)
            wbig = sb.tile([128, CP], F32)
            nc.vector.scalar_tensor_tensor(
                out=wbig, in0=prior_rep, scalar=tref, in1=b128_rep,
                op0=ALU.subtract, op1=ALU.mult,
            )
            # ones block bounced through DVE so each matmul's producers all
            # sit on the DVE semaphore: one embedded wait
            ones_blk = sb.tile([128, C], F32)
            nc.vector.tensor_copy(ones_blk, ones_in)
            # per-core probability column sums (independent: scheduled early)
            nc.vector.reduce_sum(
                out_stage[:, 0:1], pt, axis=mybir.AxisListType.X
            )

            ebig = sb.tile([128, C, PCOLS], F32)
            nc.scalar.activation(
                ebig, wbig.rearrange("p (c f) -> p c f", c=C), ACT.Exp
            )
            r_part = sb.tile([128, C], F32)
            nc.vector.reduce_sum(r_part, ebig, axis=mybir.AxisListType.X)
            a_init = ps.tile([C, 1], F32)
            nc.tensor.matmul(
                a_init, r_part, ones_blk[:, 0:1], start=True, stop=True
            )
            # PSUM->SBUF bounce on ACT: the scan's two producers (dec and
            # a_init_sb) then share the single ACT semaphore.  A DVE bounce
            # would cost the scan a same-engine RAW wait on top of the ACT
            # wait (DVE's pipeline is not interlocked), exceeding the one
            # embedded wait the encoding allows.
            a_init_sb = sb.tile([C, 1], F32)
            nc.scalar.copy(a_init_sb, a_init)

            # ---- decay factors and the excitation scan ----
            dt = sb.tile([C, CHUNK], F32)
            nc.vector.tensor_sub(dt, t_own, t_prev)
            negb = sb.tile([C, 1], F32)
            nc.vector.tensor_scalar_mul(negb, beta_col, -1.0)
            dec = sb.tile([C, CHUNK], F32)
            nc.scalar.activation(dec, dt, ACT.Exp, scale=negb)

            # base = mu + gamma*t/T computed while ACT works on dec
            baset = sb.tile([C, CHUNK], F32)
            nc.vector.tensor_scalar(
                out=baset, in0=t_own, scalar1=1.0 / T_WINDOW, scalar2=gamma_col,
                op0=ALU.mult, op1=ALU.mult,
            )
            base = sb.tile([C, CHUNK], F32)
            nc.vector.tensor_scalar(
                out=base, in0=baset, scalar1=mu_col, scalar2=None, op0=ALU.add
            )

            exc = sb.tile([C, CHUNK], F32)
            nc.vector.tensor_tensor_scan(
                exc, dec, dec, initial=a_init_sb, op0=ALU.mult, op1=ALU.add
            )

            # ---- intensities: lamb = alpha*E + base; pl = lamb*pT ----
            lamb = sb.tile([C, CHUNK], F32)
            nc.vector.scalar_tensor_tensor(
                out=lamb, in0=exc, scalar=alpha_col, in1=base,
                op0=ALU.mult, op1=ALU.add,
            )
            pl = sb.tile([C, CHUNK], F32)
            nc.vector.tensor_mul(pl, lamb, pt)

            # intensity replicated across C partitions (lhsT = C ones columns)
            # so the fused Ln accum fills a whole (C,1) output column
            inten = ps.tile([C, CHUNK], F32)
            nc.tensor.matmul(inten, ones_blk[0:C, :], pl, start=True, stop=True)

            # Ln + free-dim sum fused: accum_out[c] = sum_i log(intensity_i)
            logi = sb.tile([C, CHUNK], F32)
            ll = sb.tile([C, 1], F32)
            nc.scalar.activation(logi, inten, ACT.Ln, accum_out=ll)

            # stage remaining outputs on DVE so the single out-DMA waits on
            # one semaphore
            nc.vector.tensor_copy(out_stage[:, 1:2], exc[:, CHUNK - 1 : CHUNK])
            nc.vector.tensor_copy(out_stage[:, 2:3], ll)

            nc.gpsimd.dma_start(out=out_d.ap(), in_=out_stage)

    return nc


def get_nc():
    global _NC_CACHE
    if _NC_CACHE is None:
        _NC_CACHE = _build_nc()
    return _NC_CACHE


def make_in_maps(probability, event_times, mu, gamma, alpha_kernel, beta_kernel):
    t = np.ascontiguousarray(np.asarray(event_times, dtype=np.float32))
    p = np.ascontiguousarray(np.asarray(probability, dtype=np.float32))
    beta = np.asarray(beta_kernel, dtype=np.float32)
    alpha = np.asarray(alpha_kernel, dtype=np.float32)
    mu_ = np.asarray(mu, dtype=np.float32)
    gamma_ = np.asarray(gamma, dtype=np.float32)

    scal = np.stack([beta, alpha, mu_, gamma_], axis=1)
    b128 = np.broadcast_to(beta, (128, C))

    in_maps = []
    for k in range(NCORES):
        s = k * CHUNK
        t_own = np.broadcast_to(t[s : s + CHUNK], (C, CHUNK))
        tp = np.empty(CHUNK, np.float32)
        if k == 0:
            tp[0] = t[0] - BIG  # forces d_0 = 0: no events precede event 0
            tp[1:] = t[: CHUNK - 1]
        else:
            tp[:] = t[s - 1 : s + CHUNK - 1]
        t_prev = np.broadcast_to(tp, (C, CHUNK))
        pt = p[s : s + CHUNK, :].T

        npri = max(s - 1, 0)
        pri = np.full(PRIOR_PAD, -BIG, np.float32)
        pri[:npri] = t[:npri]
        prior_pm = pri.reshape(PCOLS, 128).T
        tref_val = t[s - 1] if k > 0 else t[0]
        tref = np.full((128, 1), tref_val, np.float32)

        ina = np.ascontiguousarray(
            np.concatenate([t_own, t_prev, pt, scal], axis=1, dtype=np.float32)
        )
        ones_c = np.ones((128, C), np.float32)
        prior_rep = np.tile(prior_pm, (1, C))                       # (128, 224)
        b128_rep = np.broadcast_to(
            np.repeat(beta, PCOLS)[None, :], (128, C * PCOLS)
        )
        inb = np.ascontiguousarray(
            np.concatenate(
                [prior_rep, tref, b128_rep, ones_c], axis=1, dtype=np.float32
            )
        )
        in_maps.append({"inA": ina, "inB": inb})
    return in_maps


def combine_outputs(results, event_times, mu, gamma, alpha_kernel, beta_kernel):
    """Host-side reduction of the per-core partial scalars (float64)."""
    t = np.asarray(event_times, dtype=np.float32)
    beta = np.asarray(beta_kernel, dtype=np.float64)
    alpha = np.asarray(alpha_kernel, dtype=np.float64)
    mu_ = np.asarray(mu, dtype=np.float64)
    gamma_ = np.asarray(gamma, dtype=np.float64)

    ll_sum = sum(float(r["out"][0, 2]) for r in results)
    psum = np.zeros(C, np.float64)
    for r in results:
        psum += r["out"][:, 0].astype(np.float64)
    elast = results[NCORES - 1]["out"][:, 1].astype(np.float64)

    ab = alpha / beta
    exp_term = ab * ((N - 1) - elast)
    t_diff = float(t[-1]) - float(t[0])
    t_sq_diff = float(t[-1]) ** 2 - float(t[0]) ** 2
    base_terms = t_diff * mu_ + t_sq_diff * gamma_ / (2.0 * T_WINDOW)
    integral_part = float(psum @ (exp_term + base_terms)) / N
    return np.float32(-(ll_sum - integral_part))


def kernel(probability, event_times, mu, gamma, alpha_kernel, beta_kernel):
    nc = get_nc()
    in_maps = make_in_maps(
        probability, event_times, mu, gamma, alpha_kernel, beta_kernel
    )
    res = run_bass_kernel_spmd(nc, in_maps, core_ids=list(range(NCORES))).results
    return combine_outputs(
        res, event_times, mu, gamma, alpha_kernel, beta_kernel
    )
